# revision 4
# baseline (speedup 1.0000x reference)
"""Trainium2 Bass kernel for a pre-LN transformer block (B=8,T=1024,C=1024,H=16,FF=4096).

Sharding: pure data-parallel over batch — B=8 equals the 8 NeuronCores, each core
runs the full block on one (T, C) slice; weights are replicated. No collectives.

v10 (final): token-half software pipeline on top of fp8e4-DoubleRow attention.
  - Attention is split into chunk phases: B0 processes all 16 heads on the
    i-chunk [0,512), B1 on [512,1024). During B1 the first token-half's
    attn-proj, LN2, and fc run as PE filler, overlapping the exp-paced
    attention tail. The MLP projection + second fc half follow.
  - Softmax-denominator broadcast via gpsimd partition_broadcast (no rank-1
    PE matmuls).
  - LayerNorm rstd = exp(-0.5*ln(var+eps)) on ACT: Ln and Exp share one
    activation table set with the attention exp, so the whole attention +
    LN region runs with zero ACT table switches; gelu is deferred (fc keeps
    pre-activations in SBUF) and applied in one table-set excursion pipelined
    one weight-group ahead of the MLP matmuls.
  - fp8 weights host-scaled by 32; rescales ride copy scales and
    scalar_tensor_tensor residual adds. MLP matmuls stay bf16 (fp8 there
    measured 1.7e-2 rel err, over the gate).
"""

import functools
import os

import ml_dtypes
import numpy as np

import concourse.bass as bass
import concourse.mybir as mybir
import concourse.tile as tile
from concourse import bacc
from concourse.bass_utils import run_bass_kernel_spmd

bf16 = ml_dtypes.bfloat16
f8e4 = ml_dtypes.float8_e4m3
FP32 = mybir.dt.float32
BF16 = mybir.dt.bfloat16
FP8 = mybir.dt.float8e4
AX = mybir.AxisListType
OP = mybir.AluOpType
AF = mybir.ActivationFunctionType
DR = mybir.MatmulPerfMode.DoubleRow

B, T, C, H = 8, 1024, 1024, 16
D = C // H          # 64
FF = 4 * C          # 4096
P = 128
NT = T // P         # 8 token tiles
NCT = C // P        # 8 channel tiles
NFT = FF // P       # 32 ff tiles

WS = 32.0           # fp8 weight scale (host folds WS into fp8 weights)

# Per i-chunk: (pair_vs, odd_vs) per j-tile pair; odd half of a pair must be
# zeroed on [pair_vs, odd_vs) since exp only writes [odd_vs, 512).
PAIR_VS = {0: [(0, 128), (256, 384)],
           1: [(0, 0), (0, 0), (0, 128), (256, 384)]}


def emit_block(nc, tc):
    x_d = nc.dram_tensor("x", [T, C], FP32, kind="ExternalInput").ap()
    wqk_d = nc.dram_tensor("wqk", [16, P, NCT, P], FP8, kind="ExternalInput").ap()
    wv_d = nc.dram_tensor("wv", [P, NCT, C], FP8, kind="ExternalInput").ap()
    wproj_d = nc.dram_tensor("wproj", [P, NCT, C], FP8, kind="ExternalInput").ap()
    wfc_d = nc.dram_tensor("wfc", [NFT, P, NCT, P], BF16, kind="ExternalInput").ap()
    wmp_d = nc.dram_tensor("wmp", [2, P, NFT, 512], BF16, kind="ExternalInput").ap()
    ident_d = nc.dram_tensor("ident", [P, P], BF16, kind="ExternalInput").ap()
    tri_d = nc.dram_tensor("tri01", [P, P], FP8, kind="ExternalInput").ap()
    out_d = nc.dram_tensor("out", [T, C], FP32, kind="ExternalOutput").ap()

    from contextlib import ExitStack
    with ExitStack() as top:
        cpool = top.enter_context(tc.tile_pool(name="const", bufs=1))
        ppool = top.enter_context(tc.tile_pool(name="persist", bufs=1))
        spool = top.enter_context(tc.tile_pool(name="stream", bufs=2))
        sm = top.enter_context(tc.tile_pool(name="small", bufs=4))
        aoT_pool = top.enter_context(tc.tile_pool(name="aoT", bufs=1))
        wp_pool = top.enter_context(tc.tile_pool(name="wproj", bufs=1))
        h2_pool = top.enter_context(tc.tile_pool(name="h2Tp", bufs=1))
        m0_pool = top.enter_context(tc.tile_pool(name="m0", bufs=1))
        cs = top.enter_context(ExitStack())
        ps_mm = cs.enter_context(tc.tile_pool(name="ps_mm", bufs=2, space="PSUM"))

        ident = cpool.tile([P, P], BF16, tag="ident")
        tri01 = cpool.tile([P, P], FP8, tag="tri01")
        zero1 = cpool.tile([P, 1], FP32, tag="zero1")
        eps1 = cpool.tile([P, 1], FP32, tag="eps1")

        x2_sb = ppool.tile([P, NT, C], FP32, tag="x2")
        aoT = aoT_pool.tile([P, NCT, T], FP8, tag="aoT")
        wp = wp_pool.tile([P, NCT, C], FP8, tag="wproj")
        h2T = h2_pool.tile([P, NCT, T], BF16, tag="h2T")
        m0 = m0_pool.tile([P, NFT, 512], BF16, tag="m0")  # fc half-0 pre-gelu

        def emit_ln(x_tile, h_out):
            """h_out = (x-mu)*rsqrt(var+eps). Sqrt lives in its own ACT table
            set — callers must batch LN emissions so Sqrts are consecutive in
            the ACT queue (each isolated Sqrt amid exps costs 2 table loads)."""
            width = x_tile.shape[-1]
            s = sm.tile([P, 1], FP32, tag="ln_s")
            ssq = sm.tile([P, 1], FP32, tag="ln_ssq")
            mu = sm.tile([P, 1], FP32, tag="ln_mu")
            var = sm.tile([P, 1], FP32, tag="ln_var")
            std = sm.tile([P, 1], FP32, tag="ln_std")
            rstd = sm.tile([P, 1], FP32, tag="ln_rstd")
            nc.vector.reduce_sum(s, x_tile, axis=AX.X)
            # sum-of-squares on GPSIMD (not ACT Square: the table pass assigns
            # Square a different set than Sqrt, costing 2 loads per LN; not
            # DVE so it runs concurrently with the reduce; and
            # tensor_tensor_reduce hard-faults on hw).
            # The elementwise out is scratch — write over h_out's storage.
            nc.vector.scalar_tensor_tensor(
                h_out[:, :width], x_tile, 1.0, x_tile,
                op0=OP.mult, op1=OP.mult, accum_out=ssq)
            nc.vector.tensor_scalar_mul(mu, s, 1.0 / width)
            nc.vector.tensor_scalar_mul(var, ssq, 1.0 / width)
            nc.vector.tensor_tensor(s, mu, mu, op=OP.mult)
            nc.vector.tensor_tensor(var, var, s, op=OP.subtract)
            nc.scalar.activation(std, var, AF.Sqrt, bias=1e-5)
            nc.vector.reciprocal(rstd, std)
            nc.vector.tensor_scalar(h_out, x_tile, scalar1=mu, scalar2=rstd,
                                    op0=OP.subtract, op1=OP.mult)

        with ExitStack() as attn_scope:
            ps_pv = attn_scope.enter_context(
                tc.tile_pool(name="ps_pv", bufs=2, space="PSUM"))
            ps_s = attn_scope.enter_context(
                tc.tile_pool(name="ps_s", bufs=4, space="PSUM"))
            qk_pool = attn_scope.enter_context(tc.tile_pool(name="qk", bufs=1))
            v_pool = attn_scope.enter_context(tc.tile_pool(name="v", bufs=1))
            hT_pool = attn_scope.enter_context(tc.tile_pool(name="hTp", bufs=1))
            wq_pool = attn_scope.enter_context(tc.tile_pool(name="wqkv", bufs=2))
            wv_pool = attn_scope.enter_context(tc.tile_pool(name="wvp", bufs=1))
            pt_pool = attn_scope.enter_context(tc.tile_pool(name="pt", bufs=14))
            rb_pool = attn_scope.enter_context(tc.tile_pool(name="rbp", bufs=2))

            qpT = qk_pool.tile([P, NCT, T], BF16, tag="qpT")  # q-proj^T (key role)
            kpT = qk_pool.tile([P, NCT, T], BF16, tag="kpT")  # k-proj^T (query role)
            VW = 128  # v padded to 128 cols: [0:D]=32x v, [D]=ones, rest zeros
            v_aug = v_pool.tile([P, NT, H, VW], FP8, tag="vaug")
            hT = hT_pool.tile([P, NCT, T], FP8, tag="hT")

            xts = []
            for tt in range(NT):
                xt = spool.tile([P, C], FP32, tag="xin", name=f"xin{tt}")
                nc.sync.dma_start(xt[:], x_d[tt * P:(tt + 1) * P, :])
                xts.append(xt)
            nc.sync.dma_start(ident[:], ident_d)
            nc.sync.dma_start(tri01[:], tri_d)
            nc.gpsimd.memset(zero1[:], 0.0)
            nc.gpsimd.memset(eps1[:], 1e-5)
            nc.const_aps.aps[(FP32, 0.0)] = zero1[:]
            nc.const_aps.aps[(FP32, 1e-5)] = eps1[:]
            # cols D..VW-1 are ALL ones: the PV matmul then replicates the
            # softmax denominator into psum rows 64..127, so the reciprocal
            # runs 64-lane-parallel and no partition broadcast is needed
            # (a [1,512] single-lane reciprocal measured 3.3us on DVE).
            nc.gpsimd.memset(v_aug[:, :, :, D:VW], 1.0)
            wv_sb = wv_pool.tile([P, NCT, C], FP8, tag="wv")
            nc.sync.dma_start(wv_sb[:], wv_d)
            nc.sync.dma_start(wp[:], wproj_d)

            def emit_qk_half(ft, tc2):
                wt = wq_pool.tile([P, NCT, P], FP8, tag="wqk",
                                  name=f"wqk{ft}_{tc2}")
                nc.sync.dma_start(wt[:], wqk_d[ft])
                dst, sc = (kpT, 1.0 / (WS * 8.0)) if ft < 8 else (qpT, 1.0 / WS)
                pm = ps_mm.tile([P, 512], FP32, tag="mm", name=f"qk{ft}_{tc2}")
                for c2 in range(NCT // 2):
                    nc.tensor.matmul(pm[:], wt[:, 2 * c2:2 * c2 + 2, :],
                                     hT[:, 2 * c2:2 * c2 + 2,
                                        tc2 * 512:(tc2 + 1) * 512],
                                     start=(c2 == 0), stop=(c2 == NCT // 2 - 1),
                                     perf_mode=DR)
                nc.scalar.activation(
                    dst[:, ft % 8, tc2 * 512:(tc2 + 1) * 512], pm[:],
                    AF.Copy, scale=sc)

            # ---- phase A: LN1 + transpose + v projection per token tile;
            # qk(tc2=0) halves ride along once hT tokens 0-511 exist ----
            ftlist = [v for hp in range(8) for v in (hp, 8 + hp)]
            for tt in range(NT):
                ht = spool.tile([P, C], BF16, tag="h")
                emit_ln(xts[tt][:], ht[:])
                for ct in range(NCT):
                    ptr = ps_s.tile([P, P], BF16, tag="smm")
                    nc.tensor.transpose(ptr[:], ht[:, ct * P:(ct + 1) * P], ident[:])
                    # ACT (idle here) does the psum->SBUF copies; Copy is a
                    # filler in every ACT table set so no load cost
                    nc.scalar.activation(hT[:, ct, tt * P:(tt + 1) * P],
                                         ptr[:], AF.Copy)
                for fc2 in range(2):
                    pm = ps_mm.tile([P, 512], FP32, tag="mm")
                    for c2 in range(NCT // 2):
                        nc.tensor.matmul(pm[:], hT[:, 2 * c2:2 * c2 + 2,
                                                    tt * P:(tt + 1) * P],
                                         wv_sb[:, 2 * c2:2 * c2 + 2,
                                               fc2 * 512:(fc2 + 1) * 512],
                                         start=(c2 == 0),
                                         stop=(c2 == NCT // 2 - 1),
                                         perf_mode=DR)
                    nc.scalar.activation(
                        v_aug[:, tt, fc2 * 8:(fc2 + 1) * 8, 0:D],
                        pm[:].rearrange("p (h d) -> p h d", d=D), AF.Copy)
                if tt >= 4:
                    for ft in ftlist[4 * (tt - 4):4 * (tt - 3)]:
                        emit_qk_half(ft, 0)

            # ---- attention helpers ----
            def alloc_pts(ic, h):
                out = []
                for p in range(2 * ic + 2):
                    t = pt_pool.tile([P, 2, 512], FP8, tag="pt",
                                     name=f"ptp{ic}_{h}_{p}")
                    vsp, vso = PAIR_VS[ic][p]
                    if vso > vsp:
                        nc.gpsimd.memset(t[:, 1, vsp:vso], 0.0)
                    out.append(t)
                return out

            def emit_S_one(ic, h, jt, pairs):
                po = (h % 2) * D
                cth = h // 2
                vs = max(0, jt * P - ic * 512)
                pm = ps_s.tile([P, 512], FP32, tag="smm", name=f"s{ic}_{h}_{jt}")
                nc.tensor.matmul(
                    pm[:, vs:512],
                    qpT[po:po + D, cth, jt * P:(jt + 1) * P],
                    kpT[po:po + D, cth, ic * 512 + vs:(ic + 1) * 512],
                    start=True, stop=True)
                pt = pairs[jt // 2]
                nc.scalar.activation(pt[:, jt % 2, vs:512], pm[:, vs:512], AF.Exp)
                if jt >= ic * 4:
                    dd = jt * P - ic * 512
                    nc.vector.tensor_tensor(
                        pt[:, jt % 2, dd:dd + P], pt[:, jt % 2, dd:dd + P],
                        tri01[:], op=OP.mult)

            def emit_PV(ic, h, pairs):
                pvT = ps_pv.tile([VW, 512], FP32, tag="pv", name=f"pv{ic}_{h}")
                n_p = 2 * ic + 2
                for p in range(n_p):
                    vsp = PAIR_VS[ic][p][0]
                    nc.tensor.matmul(pvT[:, vsp:512],
                                     v_aug[:, 2 * p:2 * p + 2, h, :],
                                     pairs[p][:, :, vsp:512],
                                     start=(p == 0), stop=(p == n_p - 1),
                                     perf_mode=DR)
                rd = rb_pool.tile([D, 512], BF16, tag="rd64")
                with nc.allow_low_precision(
                        reason="softmax denom recip feeds bf16 mult"):
                    nc.vector.reciprocal(rd[:], pvT[D:2 * D, :])
                return pvT, rd

            def emit_PV_norm(ic, h, pvT, rd):
                po = (h % 2) * D
                cth = h // 2
                nc.vector.tensor_tensor(
                    aoT[po:po + D, cth, ic * 512:(ic + 1) * 512],
                    pvT[0:D, :], rd[:], op=OP.mult)

            # ---- phase B0: chunk-0 attention; qk(tc2=1) halves as filler.
            # PV lags S by 2 steps and norm by 3 so exp always has slack ----
            pts_q = {}
            pv_q = {}
            B0_HEADS = [[2 * i, 2 * i + 1] for i in range(8)] + [[], []]
            for s, heads in enumerate(B0_HEADS):
                if s < 8:
                    emit_qk_half(ftlist[2 * s], 1)
                    emit_qk_half(ftlist[2 * s + 1], 1)
                prev = B0_HEADS[s - 1] if s >= 1 else []
                for h_pv in prev:
                    pv_q[h_pv] = emit_PV(0, h_pv, pts_q.pop(h_pv))
                for h in heads:
                    pts = alloc_pts(0, h)
                    for jt in range(4):
                        emit_S_one(0, h, jt, pts)
                    pts_q[h] = pts
                prev2 = B0_HEADS[s - 2] if s >= 2 else []
                for h_n in prev2:
                    emit_PV_norm(0, h_n, *pv_q.pop(h_n))

            # ---- phase B1: chunk-1 attention + token-half-0 proj/LN2/fc ----
            def emit_proj(tt):
                for cc2 in range(2):
                    pm = ps_mm.tile([P, 512], FP32, tag="mm",
                                    name=f"prj{tt}_{cc2}")
                    for c2 in range(NCT // 2):
                        nc.tensor.matmul(pm[:], aoT[:, 2 * c2:2 * c2 + 2,
                                                    tt * P:(tt + 1) * P],
                                         wp[:, 2 * c2:2 * c2 + 2,
                                            cc2 * 512:(cc2 + 1) * 512],
                                         start=(c2 == 0),
                                         stop=(c2 == NCT // 2 - 1),
                                         perf_mode=DR)
                    xr = spool.tile([P, 512], FP32, tag="xres")
                    nc.sync.dma_start(
                        xr[:], x_d[tt * P:(tt + 1) * P, cc2 * 512:(cc2 + 1) * 512])
                    nc.vector.scalar_tensor_tensor(
                        x2_sb[:, tt, cc2 * 512:(cc2 + 1) * 512], pm[:],
                        1.0 / (WS * WS), xr[:], op0=OP.mult, op1=OP.add)

            def emit_ln2(tt, trpool, trtag):
                h2 = spool.tile([P, C], BF16, tag="h")
                emit_ln(x2_sb[:, tt, :], h2[:])
                for ct in range(NCT):
                    ptr = trpool.tile([P, P], BF16, tag=trtag)
                    nc.tensor.transpose(ptr[:], h2[:, ct * P:(ct + 1) * P], ident[:])
                    nc.scalar.activation(h2T[:, ct, tt * P:(tt + 1) * P],
                                         ptr[:], AF.Copy)

            def emit_fc0(ft):
                wf = wf0_pool.tile([P, NCT, P], BF16, tag="wfc", name=f"wfc0_{ft}")
                nc.sync.dma_start(wf[:], wfc_d[ft])
                pm = ps_mm.tile([P, 512], FP32, tag="mm", name=f"fc0_{ft}")
                for ct in range(NCT):
                    nc.tensor.matmul(pm[:], wf[:, ct, :],
                                     h2T[:, ct, 0:512],
                                     start=(ct == 0), stop=(ct == NCT - 1))
                nc.vector.tensor_copy(out=m0[:, ft, :], in_=pm[:])

            wf0_pool = attn_scope.enter_context(tc.tile_pool(name="wfc0", bufs=2))

            # B1 head schedule: 2 heads/step while PE is thin (attention only),
            # 1 head/step once fc chains fill the PE. proj tts 0-3 at steps
            # 0-3, LN2 burst at 4, fc half-0 chains 3/step from step 6.
            HEADS_OF = [[0, 1], [2, 3], [4, 5], [6, 7]] + \
                [[8 + i] for i in range(8)] + [[], [], []]
            fc_ft = iter(range(NFT))
            for s, heads in enumerate(HEADS_OF):
                prev = HEADS_OF[s - 1] if s >= 1 else []
                for h_pv in prev:
                    pv_q[h_pv] = emit_PV(1, h_pv, pts_q.pop(h_pv))
                fillers = []
                if 0 <= s <= 3:
                    fillers.append(('proj', s))
                if s >= 6:
                    for _ in range(3):
                        ft = next(fc_ft, None)
                        if ft is not None:
                            fillers.append(('fc', ft))
                fi = iter(fillers)

                def emit_filler(f):
                    kind, a = f
                    if kind == 'proj':
                        emit_proj(a)
                    else:
                        emit_fc0(a)

                for h in heads:
                    pts = alloc_pts(1, h)
                    for jt in range(8):
                        emit_S_one(1, h, jt, pts)
                        if jt % 3 == 2:
                            f = next(fi, None)
                            if f is not None:
                                emit_filler(f)
                    pts_q[h] = pts
                for f in fi:
                    emit_filler(f)
                if s == 4:
                    # all 4 LN2s in one burst at a step boundary: Sqrts stay
                    # consecutive in the ACT queue (2 table transitions) and
                    # don't split a head's S/exp stream
                    for tt2 in range(4):
                        emit_ln2(tt2, ps_s, "smm")
                prev2 = HEADS_OF[s - 2] if s >= 2 else []
                for h_n in prev2:
                    emit_PV_norm(1, h_n, *pv_q.pop(h_n))
            for ft in fc_ft:
                emit_fc0(ft)

        # ---- post: proj/LN2 tokens 4-7, then fc half 1 OVERLAPPED with the
        # MLP projection for token half 0 (which only needs m0); MLP half 1
        # finishes in a single ft sweep with all 8 PSUM banks ----
        if True:
            ps_fc = cs.enter_context(
                tc.tile_pool(name="ps_fc", bufs=2, space="PSUM"))
            ps_ml0 = cs.enter_context(
                tc.tile_pool(name="ps_ml0", bufs=4, space="PSUM"))
            m1_pool = cs.enter_context(tc.tile_pool(name="m1", bufs=1))
            wf_pool = cs.enter_context(tc.tile_pool(name="wfc", bufs=3))
            wm0_pool = cs.enter_context(tc.tile_pool(name="wmp0", bufs=3))
            m1 = m1_pool.tile([P, NFT, 512], BF16, tag="m1")

            # LN2 transposes for tokens 4-7 share ps_mm's bank slots
            for tt in range(4, NT):
                emit_proj(tt)
                emit_ln2(tt, ps_mm, "mm")

            # gate: 1.0, but data-dependent on the last chunk-1 norms so the
            # greedy scheduler cannot hoist the m0 gelus (and their two ACT
            # table loads) into attention-phase exp bubbles.
            gate = cpool.tile([P, 1], FP32, tag="gate")
            nc.vector.tensor_scalar(gate[:], aoT[:, NCT - 1, T - 1:T],
                                    scalar1=0.0, scalar2=1.0,
                                    op0=OP.mult, op1=OP.add)
            for fg in range(NFT // 4):
                nc.scalar.activation(m0[:, 4 * fg:4 * fg + 4, :],
                                     m0[:, 4 * fg:4 * fg + 4, :], AF.Gelu,
                                     scale=gate[:])

            def emit_fc1(ft):
                wf = wf_pool.tile([P, NCT, P], BF16, tag="wfc", name=f"wfc1_{ft}")
                nc.sync.dma_start(wf[:], wfc_d[ft])
                pm = ps_fc.tile([P, 512], FP32, tag="fcp", name=f"fc1_{ft}")
                for ct in range(NCT):
                    nc.tensor.matmul(pm[:], wf[:, ct, :],
                                     h2T[:, ct, 512:1024],
                                     start=(ct == 0), stop=(ct == NCT - 1))
                nc.vector.tensor_copy(out=m1[:, ft, :], in_=pm[:])

            # fc half-1 chains interleaved with MLP-proj for token half 0.
            # mlp0 runs cc2 passes sequentially on 4 banks; 2 fc1 chains and
            # (during cc2=0) one m1 gelu group ride along per weight group.
            fc1_ft = iter(range(NFT))
            for cc2 in range(2):
                pms = [ps_ml0.tile([P, 512], FP32, tag="ml0",
                                   name=f"ml0_{cc2}_{i}") for i in range(4)]
                for fg in range(NFT // 4):
                    for _ in range(2):
                        ft1 = next(fc1_ft, None)
                        if ft1 is not None:
                            emit_fc1(ft1)
                            if ft1 % 4 == 3:
                                g = ft1 // 4
                                nc.scalar.activation(
                                    m1[:, 4 * g:4 * g + 4, :],
                                    m1[:, 4 * g:4 * g + 4, :], AF.Gelu)
                    wm = wm0_pool.tile([P, 4, 512], BF16, tag="wmp0")
                    nc.sync.dma_start(wm[:], wmp_d[cc2][:, fg * 4:(fg + 1) * 4, :])
                    for fi in range(4):
                        ft = fg * 4 + fi
                        for tt in range(4):
                            nc.tensor.matmul(pms[tt][:],
                                             m0[:, ft, tt * P:(tt + 1) * P],
                                             wm[:, fi, :],
                                             start=(ft == 0),
                                             stop=(ft == NFT - 1))
                for tt in range(4):
                    ot = spool.tile([P, 512], FP32, tag="osb")
                    nc.vector.tensor_tensor(
                        ot[:], pms[tt][:], x2_sb[:, tt, cc2 * 512:(cc2 + 1) * 512],
                        op=OP.add)
                    nc.sync.dma_start(
                        out_d[tt * P:(tt + 1) * P, cc2 * 512:(cc2 + 1) * 512],
                        ot[:])
            for ft1 in fc1_ft:
                emit_fc1(ft1)
                if ft1 % 4 == 3:
                    g = ft1 // 4
                    nc.scalar.activation(m1[:, 4 * g:4 * g + 4, :],
                                         m1[:, 4 * g:4 * g + 4, :], AF.Gelu)

            # MLP-proj token half 1 on the same 4-bank pool (a scope close
            # here costs a full-engine drain barrier mid-MLP)
            for cc2 in range(2):
                pms = [ps_ml0.tile([P, 512], FP32, tag="ml0",
                                   name=f"ml1_{cc2}_{i}") for i in range(4)]
                for fg in range(NFT // 4):
                    wm = wm0_pool.tile([P, 4, 512], BF16, tag="wmp0",
                                       name=f"wm1_{cc2}_{fg}")
                    nc.sync.dma_start(wm[:], wmp_d[cc2][:, fg * 4:(fg + 1) * 4, :])
                    for fi in range(4):
                        ft = fg * 4 + fi
                        for tt in range(4):
                            nc.tensor.matmul(pms[tt][:],
                                             m1[:, ft, tt * P:(tt + 1) * P],
                                             wm[:, fi, :],
                                             start=(ft == 0),
                                             stop=(ft == NFT - 1))
                for tt in range(4):
                    ot = spool.tile([P, 512], FP32, tag="osb")
                    nc.vector.tensor_tensor(
                        ot[:], pms[tt][:],
                        x2_sb[:, tt + 4, cc2 * 512:(cc2 + 1) * 512],
                        op=OP.add)
                    nc.sync.dma_start(
                        out_d[(tt + 4) * P:(tt + 5) * P,
                              cc2 * 512:(cc2 + 1) * 512], ot[:])


@functools.lru_cache(maxsize=1)
def _compiled():
    nc = bacc.Bacc("TRN2", target_bir_lowering=False, debug=False)
    with tile.TileContext(nc) as tc:
        emit_block(nc, tc)
    nc.compile()
    return nc


def _prepro(inputs):
    f32 = np.float32
    inp = {k: np.asarray(v, f32) for k, v in inputs.items()}
    g1, b1 = inp["ln1_g"], inp["ln1_b"]
    W = inp["attn_w"] * g1[:, None]
    bias_kqv = inp["attn_b"] + b1 @ inp["attn_w"]
    assert not np.any(bias_kqv), "nonzero attn bias not supported by this build"
    assert not np.any(inp["attn_proj_b"]) and not np.any(inp["fc_b"]) \
        and not np.any(inp["mlp_proj_b"]), "nonzero biases not supported"

    def to_f8(a):
        return np.clip(a * WS, -240.0, 240.0).astype(f8e4)

    wqk = np.ascontiguousarray(
        to_f8(W[:, :2 * C]).reshape(NCT, P, 16, P).transpose(2, 1, 0, 3))
    wv = np.ascontiguousarray(
        to_f8(W[:, 2 * C:]).reshape(NCT, P, C).transpose(1, 0, 2))
    wproj = np.ascontiguousarray(
        to_f8(inp["attn_proj_w"]).reshape(NCT, P, C).transpose(1, 0, 2))
    wfc = np.ascontiguousarray(
        (inp["fc_w"] * inp["ln2_g"][:, None]).astype(bf16)
        .reshape(NCT, P, NFT, P).transpose(2, 1, 0, 3))
    assert not np.any(inp["ln2_b"]), "nonzero ln2 bias not supported"
    wmp = np.ascontiguousarray(
        inp["mlp_proj_w"].astype(bf16).reshape(NFT, P, 2, 512).transpose(2, 1, 0, 3))
    ident = np.eye(P, dtype=bf16)
    tri01 = np.triu(np.ones((P, P), np.float32)).astype(f8e4)  # 1 where col >= row
    return inp["x"], dict(wqk=wqk, wv=wv, wproj=wproj, wfc=wfc, wmp=wmp,
                          ident=ident, tri01=tri01)


def kernel(**inputs) -> np.ndarray:
    x, weights = _prepro(inputs)
    nc = _compiled()
    in_maps = [{"x": np.ascontiguousarray(x[b]), **weights} for b in range(B)]
    res = run_bass_kernel_spmd(nc, in_maps, list(range(B)))
    return np.stack([res.results[b]["out"] for b in range(B)]).astype(np.float32)


# revision 5
# speedup vs baseline: 1.0350x; 1.0350x over previous
"""Trainium2 Bass kernel for a pre-LN transformer block (B=8,T=1024,C=1024,H=16,FF=4096).

Sharding: pure data-parallel over batch — B=8 equals the 8 NeuronCores, each core
runs the full block on one (T, C) slice; weights are replicated. No collectives.

v10 (final): token-half software pipeline on top of fp8e4-DoubleRow attention.
  - Attention is split into chunk phases: B0 processes all 16 heads on the
    i-chunk [0,512), B1 on [512,1024). During B1 the first token-half's
    attn-proj, LN2, and fc run as PE filler, overlapping the exp-paced
    attention tail. The MLP projection + second fc half follow.
  - Softmax-denominator broadcast via gpsimd partition_broadcast (no rank-1
    PE matmuls).
  - LayerNorm rstd = exp(-0.5*ln(var+eps)) on ACT: Ln and Exp share one
    activation table set with the attention exp, so the whole attention +
    LN region runs with zero ACT table switches; gelu is deferred (fc keeps
    pre-activations in SBUF) and applied in one table-set excursion pipelined
    one weight-group ahead of the MLP matmuls.
  - fp8 weights host-scaled by 32; rescales ride copy scales and
    scalar_tensor_tensor residual adds. MLP matmuls stay bf16 (fp8 there
    measured 1.7e-2 rel err, over the gate).
"""

import functools
import os

import ml_dtypes
import numpy as np

import concourse.bass as bass
import concourse.mybir as mybir
import concourse.tile as tile
from concourse import bacc
from concourse.bass_utils import run_bass_kernel_spmd

bf16 = ml_dtypes.bfloat16
f8e4 = ml_dtypes.float8_e4m3
FP32 = mybir.dt.float32
BF16 = mybir.dt.bfloat16
FP8 = mybir.dt.float8e4
AX = mybir.AxisListType
OP = mybir.AluOpType
AF = mybir.ActivationFunctionType
DR = mybir.MatmulPerfMode.DoubleRow

B, T, C, H = 8, 1024, 1024, 16
D = C // H          # 64
FF = 4 * C          # 4096
P = 128
NT = T // P         # 8 token tiles
NCT = C // P        # 8 channel tiles
NFT = FF // P       # 32 ff tiles

WS = 32.0           # fp8 weight scale (host folds WS into fp8 weights)

# Per i-chunk: (pair_vs, odd_vs) per j-tile pair; odd half of a pair must be
# zeroed on [pair_vs, odd_vs) since exp only writes [odd_vs, 512).
PAIR_VS = {0: [(0, 128), (256, 384)],
           1: [(0, 0), (0, 0), (0, 128), (256, 384)]}


def emit_block(nc, tc):
    x_d = nc.dram_tensor("x", [T, C], FP32, kind="ExternalInput").ap()
    wqk_d = nc.dram_tensor("wqk", [16, P, NCT, P], FP8, kind="ExternalInput").ap()
    wv_d = nc.dram_tensor("wv", [P, NCT, C], FP8, kind="ExternalInput").ap()
    wproj_d = nc.dram_tensor("wproj", [P, NCT, C], FP8, kind="ExternalInput").ap()
    wfc_d = nc.dram_tensor("wfc", [NFT, P, NCT, P], BF16, kind="ExternalInput").ap()
    wmp_d = nc.dram_tensor("wmp", [2, P, NFT, 512], BF16, kind="ExternalInput").ap()
    ident_d = nc.dram_tensor("ident", [P, P], BF16, kind="ExternalInput").ap()
    tri_d = nc.dram_tensor("tri01", [P, P], FP8, kind="ExternalInput").ap()
    out_d = nc.dram_tensor("out", [T, C], FP32, kind="ExternalOutput").ap()

    from contextlib import ExitStack
    with ExitStack() as top:
        cpool = top.enter_context(tc.tile_pool(name="const", bufs=1))
        ppool = top.enter_context(tc.tile_pool(name="persist", bufs=1))
        spool = top.enter_context(tc.tile_pool(name="stream", bufs=2))
        sm = top.enter_context(tc.tile_pool(name="small", bufs=4))
        aoT_pool = top.enter_context(tc.tile_pool(name="aoT", bufs=1))
        wp_pool = top.enter_context(tc.tile_pool(name="wproj", bufs=1))
        h2_pool = top.enter_context(tc.tile_pool(name="h2Tp", bufs=1))
        m0_pool = top.enter_context(tc.tile_pool(name="m0", bufs=1))
        cs = top.enter_context(ExitStack())
        ps_mm = cs.enter_context(tc.tile_pool(name="ps_mm", bufs=2, space="PSUM"))

        ident = cpool.tile([P, P], BF16, tag="ident")
        tri01 = cpool.tile([P, P], FP8, tag="tri01")
        zero1 = cpool.tile([P, 1], FP32, tag="zero1")
        eps1 = cpool.tile([P, 1], FP32, tag="eps1")

        x2_sb = ppool.tile([P, NT, C], FP32, tag="x2")
        aoT = aoT_pool.tile([P, NCT, T], FP8, tag="aoT")
        wp = wp_pool.tile([P, NCT, C], FP8, tag="wproj")
        h2T = h2_pool.tile([P, NCT, T], BF16, tag="h2T")
        m0 = m0_pool.tile([P, NFT, 512], BF16, tag="m0")  # fc half-0 pre-gelu

        def emit_ln(x_tile, h_out):
            """h_out = (x-mu)*rsqrt(var+eps). Sqrt lives in its own ACT table
            set — callers must batch LN emissions so Sqrts are consecutive in
            the ACT queue (each isolated Sqrt amid exps costs 2 table loads)."""
            width = x_tile.shape[-1]
            s = sm.tile([P, 1], FP32, tag="ln_s")
            ssq = sm.tile([P, 1], FP32, tag="ln_ssq")
            mu = sm.tile([P, 1], FP32, tag="ln_mu")
            var = sm.tile([P, 1], FP32, tag="ln_var")
            std = sm.tile([P, 1], FP32, tag="ln_std")
            rstd = sm.tile([P, 1], FP32, tag="ln_rstd")
            nc.vector.reduce_sum(s, x_tile, axis=AX.X)
            # sum-of-squares on GPSIMD (not ACT Square: the table pass assigns
            # Square a different set than Sqrt, costing 2 loads per LN; not
            # DVE so it runs concurrently with the reduce; and
            # tensor_tensor_reduce hard-faults on hw).
            # The elementwise out is scratch — write over h_out's storage.
            nc.vector.scalar_tensor_tensor(
                h_out[:, :width], x_tile, 1.0, x_tile,
                op0=OP.mult, op1=OP.mult, accum_out=ssq)
            nc.vector.tensor_scalar_mul(mu, s, 1.0 / width)
            nc.vector.tensor_scalar_mul(var, ssq, 1.0 / width)
            nc.vector.tensor_tensor(s, mu, mu, op=OP.mult)
            nc.vector.tensor_tensor(var, var, s, op=OP.subtract)
            nc.scalar.activation(std, var, AF.Sqrt, bias=1e-5)
            nc.vector.reciprocal(rstd, std)
            nc.vector.tensor_scalar(h_out, x_tile, scalar1=mu, scalar2=rstd,
                                    op0=OP.subtract, op1=OP.mult)

        with ExitStack() as attn_scope:
            ps_pv = attn_scope.enter_context(
                tc.tile_pool(name="ps_pv", bufs=2, space="PSUM"))
            ps_s = attn_scope.enter_context(
                tc.tile_pool(name="ps_s", bufs=4, space="PSUM"))
            qk_pool = attn_scope.enter_context(tc.tile_pool(name="qk", bufs=1))
            v_pool = attn_scope.enter_context(tc.tile_pool(name="v", bufs=1))
            hT_pool = attn_scope.enter_context(tc.tile_pool(name="hTp", bufs=1))
            wq_pool = attn_scope.enter_context(tc.tile_pool(name="wqkv", bufs=2))
            wv_pool = attn_scope.enter_context(tc.tile_pool(name="wvp", bufs=1))
            pt_pool = attn_scope.enter_context(tc.tile_pool(name="pt", bufs=14))
            rb_pool = attn_scope.enter_context(tc.tile_pool(name="rbp", bufs=2))

            qpT = qk_pool.tile([P, NCT, T], BF16, tag="qpT")  # q-proj^T (key role)
            kpT = qk_pool.tile([P, NCT, T], BF16, tag="kpT")  # k-proj^T (query role)
            VW = 128  # v padded to 128 cols: [0:D]=32x v, [D]=ones, rest zeros
            v_aug = v_pool.tile([P, NT, H, VW], FP8, tag="vaug")
            hT = hT_pool.tile([P, NCT, T], FP8, tag="hT")

            xts = []
            for tt in range(NT):
                xt = spool.tile([P, C], FP32, tag="xin", name=f"xin{tt}")
                nc.sync.dma_start(xt[:], x_d[tt * P:(tt + 1) * P, :])
                xts.append(xt)
            nc.sync.dma_start(ident[:], ident_d)
            nc.sync.dma_start(tri01[:], tri_d)
            nc.gpsimd.memset(zero1[:], 0.0)
            nc.gpsimd.memset(eps1[:], 1e-5)
            nc.const_aps.aps[(FP32, 0.0)] = zero1[:]
            nc.const_aps.aps[(FP32, 1e-5)] = eps1[:]
            # cols D..VW-1 are ALL ones: the PV matmul then replicates the
            # softmax denominator into psum rows 64..127, so the reciprocal
            # runs 64-lane-parallel and no partition broadcast is needed
            # (a [1,512] single-lane reciprocal measured 3.3us on DVE).
            nc.gpsimd.memset(v_aug[:, :, :, D:VW], 1.0)
            wv_sb = wv_pool.tile([P, NCT, C], FP8, tag="wv")
            nc.sync.dma_start(wv_sb[:], wv_d)
            nc.sync.dma_start(wp[:], wproj_d)

            def emit_qk_half(ft, tc2):
                wt = wq_pool.tile([P, NCT, P], FP8, tag="wqk",
                                  name=f"wqk{ft}_{tc2}")
                nc.sync.dma_start(wt[:], wqk_d[ft])
                dst, sc = (kpT, 1.0 / (WS * 8.0)) if ft < 8 else (qpT, 1.0 / WS)
                pm = ps_mm.tile([P, 512], FP32, tag="mm", name=f"qk{ft}_{tc2}")
                for c2 in range(NCT // 2):
                    nc.tensor.matmul(pm[:], wt[:, 2 * c2:2 * c2 + 2, :],
                                     hT[:, 2 * c2:2 * c2 + 2,
                                        tc2 * 512:(tc2 + 1) * 512],
                                     start=(c2 == 0), stop=(c2 == NCT // 2 - 1),
                                     perf_mode=DR)
                nc.scalar.activation(
                    dst[:, ft % 8, tc2 * 512:(tc2 + 1) * 512], pm[:],
                    AF.Copy, scale=sc)

            # ---- phase A: LN1 + transpose + v projection per token tile;
            # qk(tc2=0) halves ride along once hT tokens 0-511 exist ----
            ftlist = [v for hp in range(8) for v in (hp, 8 + hp)]
            for tt in range(NT):
                ht = spool.tile([P, C], BF16, tag="h")
                emit_ln(xts[tt][:], ht[:])
                for ct in range(NCT):
                    ptr = ps_s.tile([P, P], BF16, tag="smm")
                    nc.tensor.transpose(ptr[:], ht[:, ct * P:(ct + 1) * P], ident[:])
                    # ACT (idle here) does the psum->SBUF copies; Copy is a
                    # filler in every ACT table set so no load cost
                    nc.scalar.activation(hT[:, ct, tt * P:(tt + 1) * P],
                                         ptr[:], AF.Copy)
                for fc2 in range(2):
                    pm = ps_mm.tile([P, 512], FP32, tag="mm")
                    for c2 in range(NCT // 2):
                        nc.tensor.matmul(pm[:], hT[:, 2 * c2:2 * c2 + 2,
                                                    tt * P:(tt + 1) * P],
                                         wv_sb[:, 2 * c2:2 * c2 + 2,
                                               fc2 * 512:(fc2 + 1) * 512],
                                         start=(c2 == 0),
                                         stop=(c2 == NCT // 2 - 1),
                                         perf_mode=DR)
                    nc.scalar.activation(
                        v_aug[:, tt, fc2 * 8:(fc2 + 1) * 8, 0:D],
                        pm[:].rearrange("p (h d) -> p h d", d=D), AF.Copy)
                if tt >= 4:
                    for ft in ftlist[4 * (tt - 4):4 * (tt - 3)]:
                        emit_qk_half(ft, 0)

            # ---- attention helpers ----
            def alloc_pts(ic, h):
                out = []
                for p in range(2 * ic + 2):
                    t = pt_pool.tile([P, 2, 512], FP8, tag="pt",
                                     name=f"ptp{ic}_{h}_{p}")
                    vsp, vso = PAIR_VS[ic][p]
                    if vso > vsp:
                        nc.gpsimd.memset(t[:, 1, vsp:vso], 0.0)
                    out.append(t)
                return out

            def emit_S_one(ic, h, jt, pairs):
                po = (h % 2) * D
                cth = h // 2
                vs = max(0, jt * P - ic * 512)
                pm = ps_s.tile([P, 512], FP32, tag="smm", name=f"s{ic}_{h}_{jt}")
                nc.tensor.matmul(
                    pm[:, vs:512],
                    qpT[po:po + D, cth, jt * P:(jt + 1) * P],
                    kpT[po:po + D, cth, ic * 512 + vs:(ic + 1) * 512],
                    start=True, stop=True)
                pt = pairs[jt // 2]
                nc.scalar.activation(pt[:, jt % 2, vs:512], pm[:, vs:512], AF.Exp)
                if jt >= ic * 4:
                    dd = jt * P - ic * 512
                    # causal mask on gpsimd: pure-SBUF op, keeps it off the
                    # DVE which paces the attention phases
                    nc.gpsimd.tensor_tensor(
                        pt[:, jt % 2, dd:dd + P], pt[:, jt % 2, dd:dd + P],
                        tri01[:], op=OP.mult)

            def emit_PV(ic, h, pairs):
                pvT = ps_pv.tile([VW, 512], FP32, tag="pv", name=f"pv{ic}_{h}")
                n_p = 2 * ic + 2
                for p in range(n_p):
                    vsp = PAIR_VS[ic][p][0]
                    nc.tensor.matmul(pvT[:, vsp:512],
                                     v_aug[:, 2 * p:2 * p + 2, h, :],
                                     pairs[p][:, :, vsp:512],
                                     start=(p == 0), stop=(p == n_p - 1),
                                     perf_mode=DR)
                rd = rb_pool.tile([D, 512], BF16, tag="rd64")
                with nc.allow_low_precision(
                        reason="softmax denom recip feeds bf16 mult"):
                    nc.vector.reciprocal(rd[:], pvT[D:2 * D, :])
                return pvT, rd

            def emit_PV_norm(ic, h, pvT, rd):
                po = (h % 2) * D
                cth = h // 2
                nc.vector.tensor_tensor(
                    aoT[po:po + D, cth, ic * 512:(ic + 1) * 512],
                    pvT[0:D, :], rd[:], op=OP.mult)

            # ---- phase B0: chunk-0 attention; qk(tc2=1) halves as filler.
            # PV lags S by 2 steps and norm by 3 so exp always has slack ----
            pts_q = {}
            pv_q = {}
            B0_HEADS = [[2 * i, 2 * i + 1] for i in range(8)] + [[], []]
            for s, heads in enumerate(B0_HEADS):
                if s < 8:
                    emit_qk_half(ftlist[2 * s], 1)
                    emit_qk_half(ftlist[2 * s + 1], 1)
                prev = B0_HEADS[s - 1] if s >= 1 else []
                for h_pv in prev:
                    pv_q[h_pv] = emit_PV(0, h_pv, pts_q.pop(h_pv))
                for h in heads:
                    pts = alloc_pts(0, h)
                    for jt in range(4):
                        emit_S_one(0, h, jt, pts)
                    pts_q[h] = pts
                prev2 = B0_HEADS[s - 2] if s >= 2 else []
                for h_n in prev2:
                    emit_PV_norm(0, h_n, *pv_q.pop(h_n))

            # ---- phase B1: chunk-1 attention + token-half-0 proj/LN2/fc ----
            def emit_proj(tt):
                for cc2 in range(2):
                    pm = ps_mm.tile([P, 512], FP32, tag="mm",
                                    name=f"prj{tt}_{cc2}")
                    for c2 in range(NCT // 2):
                        nc.tensor.matmul(pm[:], aoT[:, 2 * c2:2 * c2 + 2,
                                                    tt * P:(tt + 1) * P],
                                         wp[:, 2 * c2:2 * c2 + 2,
                                            cc2 * 512:(cc2 + 1) * 512],
                                         start=(c2 == 0),
                                         stop=(c2 == NCT // 2 - 1),
                                         perf_mode=DR)
                    xr = spool.tile([P, 512], FP32, tag="xres")
                    nc.sync.dma_start(
                        xr[:], x_d[tt * P:(tt + 1) * P, cc2 * 512:(cc2 + 1) * 512])
                    nc.vector.scalar_tensor_tensor(
                        x2_sb[:, tt, cc2 * 512:(cc2 + 1) * 512], pm[:],
                        1.0 / (WS * WS), xr[:], op0=OP.mult, op1=OP.add)

            def emit_ln2(tt, trpool, trtag):
                h2 = spool.tile([P, C], BF16, tag="h")
                emit_ln(x2_sb[:, tt, :], h2[:])
                for ct in range(NCT):
                    ptr = trpool.tile([P, P], BF16, tag=trtag)
                    nc.tensor.transpose(ptr[:], h2[:, ct * P:(ct + 1) * P], ident[:])
                    nc.scalar.activation(h2T[:, ct, tt * P:(tt + 1) * P],
                                         ptr[:], AF.Copy)

            def emit_fc0(ft):
                wf = wf0_pool.tile([P, NCT, P], BF16, tag="wfc", name=f"wfc0_{ft}")
                nc.sync.dma_start(wf[:], wfc_d[ft])
                pm = ps_mm.tile([P, 512], FP32, tag="mm", name=f"fc0_{ft}")
                for ct in range(NCT):
                    nc.tensor.matmul(pm[:], wf[:, ct, :],
                                     h2T[:, ct, 0:512],
                                     start=(ct == 0), stop=(ct == NCT - 1))
                nc.vector.tensor_copy(out=m0[:, ft, :], in_=pm[:])

            wf0_pool = attn_scope.enter_context(tc.tile_pool(name="wfc0", bufs=2))

            # B1 head schedule: 2 heads/step while PE is thin (attention only),
            # 1 head/step once fc chains fill the PE. proj tts 0-3 at steps
            # 0-3, LN2 burst at 4, fc half-0 chains 3/step from step 6.
            HEADS_OF = [[0, 1], [2, 3], [4, 5], [6, 7]] + \
                [[8 + i] for i in range(8)] + [[], [], []]
            fc_ft = iter(range(NFT))
            for s, heads in enumerate(HEADS_OF):
                prev = HEADS_OF[s - 1] if s >= 1 else []
                for h_pv in prev:
                    pv_q[h_pv] = emit_PV(1, h_pv, pts_q.pop(h_pv))
                fillers = []
                if 0 <= s <= 3:
                    fillers.append(('proj', s))
                if s >= 6:
                    for _ in range(3):
                        ft = next(fc_ft, None)
                        if ft is not None:
                            fillers.append(('fc', ft))
                fi = iter(fillers)

                def emit_filler(f):
                    kind, a = f
                    if kind == 'proj':
                        emit_proj(a)
                    else:
                        emit_fc0(a)

                for h in heads:
                    pts = alloc_pts(1, h)
                    for jt in range(8):
                        emit_S_one(1, h, jt, pts)
                        if jt % 3 == 2:
                            f = next(fi, None)
                            if f is not None:
                                emit_filler(f)
                    pts_q[h] = pts
                for f in fi:
                    emit_filler(f)
                if s == 4:
                    # all 4 LN2s in one burst at a step boundary: Sqrts stay
                    # consecutive in the ACT queue (2 table transitions) and
                    # don't split a head's S/exp stream
                    for tt2 in range(4):
                        emit_ln2(tt2, ps_s, "smm")
                prev2 = HEADS_OF[s - 2] if s >= 2 else []
                for h_n in prev2:
                    emit_PV_norm(1, h_n, *pv_q.pop(h_n))
            for ft in fc_ft:
                emit_fc0(ft)

        # ---- post: proj/LN2 tokens 4-7, then fc half 1 OVERLAPPED with the
        # MLP projection for token half 0 (which only needs m0); MLP half 1
        # finishes in a single ft sweep with all 8 PSUM banks ----
        if True:
            ps_fc = cs.enter_context(
                tc.tile_pool(name="ps_fc", bufs=2, space="PSUM"))
            ps_ml0 = cs.enter_context(
                tc.tile_pool(name="ps_ml0", bufs=4, space="PSUM"))
            m1_pool = cs.enter_context(tc.tile_pool(name="m1", bufs=1))
            wf_pool = cs.enter_context(tc.tile_pool(name="wfc", bufs=3))
            wm0_pool = cs.enter_context(tc.tile_pool(name="wmp0", bufs=3))
            m1 = m1_pool.tile([P, NFT, 512], BF16, tag="m1")

            # LN2 transposes for tokens 4-7 share ps_mm's bank slots
            for tt in range(4, NT):
                emit_proj(tt)
                emit_ln2(tt, ps_mm, "mm")

            # gate: 1.0, but data-dependent on the last chunk-1 norms so the
            # greedy scheduler cannot hoist the m0 gelus (and their two ACT
            # table loads) into attention-phase exp bubbles.
            gate = cpool.tile([P, 1], FP32, tag="gate")
            nc.vector.tensor_scalar(gate[:], aoT[:, NCT - 1, T - 1:T],
                                    scalar1=0.0, scalar2=1.0,
                                    op0=OP.mult, op1=OP.add)
            for fg in range(NFT // 4):
                nc.scalar.activation(m0[:, 4 * fg:4 * fg + 4, :],
                                     m0[:, 4 * fg:4 * fg + 4, :], AF.Gelu,
                                     scale=gate[:])

            def emit_fc1(ft):
                wf = wf_pool.tile([P, NCT, P], BF16, tag="wfc", name=f"wfc1_{ft}")
                nc.sync.dma_start(wf[:], wfc_d[ft])
                pm = ps_fc.tile([P, 512], FP32, tag="fcp", name=f"fc1_{ft}")
                for ct in range(NCT):
                    nc.tensor.matmul(pm[:], wf[:, ct, :],
                                     h2T[:, ct, 512:1024],
                                     start=(ct == 0), stop=(ct == NCT - 1))
                nc.vector.tensor_copy(out=m1[:, ft, :], in_=pm[:])

            # fc half-1 chains interleaved with MLP-proj for token half 0.
            # mlp0 runs cc2 passes sequentially on 4 banks; 2 fc1 chains and
            # (during cc2=0) one m1 gelu group ride along per weight group.
            fc1_ft = iter(range(NFT))
            for cc2 in range(2):
                pms = [ps_ml0.tile([P, 512], FP32, tag="ml0",
                                   name=f"ml0_{cc2}_{i}") for i in range(4)]
                for fg in range(NFT // 4):
                    for _ in range(2):
                        ft1 = next(fc1_ft, None)
                        if ft1 is not None:
                            emit_fc1(ft1)
                            if ft1 % 4 == 3:
                                g = ft1 // 4
                                nc.scalar.activation(
                                    m1[:, 4 * g:4 * g + 4, :],
                                    m1[:, 4 * g:4 * g + 4, :], AF.Gelu)
                    wm = wm0_pool.tile([P, 4, 512], BF16, tag="wmp0")
                    nc.sync.dma_start(wm[:], wmp_d[cc2][:, fg * 4:(fg + 1) * 4, :])
                    for fi in range(4):
                        ft = fg * 4 + fi
                        for tt in range(4):
                            nc.tensor.matmul(pms[tt][:],
                                             m0[:, ft, tt * P:(tt + 1) * P],
                                             wm[:, fi, :],
                                             start=(ft == 0),
                                             stop=(ft == NFT - 1))
                for tt in range(4):
                    ot = spool.tile([P, 512], FP32, tag="osb")
                    nc.vector.tensor_tensor(
                        ot[:], pms[tt][:], x2_sb[:, tt, cc2 * 512:(cc2 + 1) * 512],
                        op=OP.add)
                    nc.sync.dma_start(
                        out_d[tt * P:(tt + 1) * P, cc2 * 512:(cc2 + 1) * 512],
                        ot[:])
            for ft1 in fc1_ft:
                emit_fc1(ft1)
                if ft1 % 4 == 3:
                    g = ft1 // 4
                    nc.scalar.activation(m1[:, 4 * g:4 * g + 4, :],
                                         m1[:, 4 * g:4 * g + 4, :], AF.Gelu)

            # MLP-proj token half 1 on the same 4-bank pool (a scope close
            # here costs a full-engine drain barrier mid-MLP)
            for cc2 in range(2):
                pms = [ps_ml0.tile([P, 512], FP32, tag="ml0",
                                   name=f"ml1_{cc2}_{i}") for i in range(4)]
                for fg in range(NFT // 4):
                    wm = wm0_pool.tile([P, 4, 512], BF16, tag="wmp0",
                                       name=f"wm1_{cc2}_{fg}")
                    nc.sync.dma_start(wm[:], wmp_d[cc2][:, fg * 4:(fg + 1) * 4, :])
                    for fi in range(4):
                        ft = fg * 4 + fi
                        for tt in range(4):
                            nc.tensor.matmul(pms[tt][:],
                                             m1[:, ft, tt * P:(tt + 1) * P],
                                             wm[:, fi, :],
                                             start=(ft == 0),
                                             stop=(ft == NFT - 1))
                for tt in range(4):
                    ot = spool.tile([P, 512], FP32, tag="osb")
                    nc.vector.tensor_tensor(
                        ot[:], pms[tt][:],
                        x2_sb[:, tt + 4, cc2 * 512:(cc2 + 1) * 512],
                        op=OP.add)
                    nc.sync.dma_start(
                        out_d[(tt + 4) * P:(tt + 5) * P,
                              cc2 * 512:(cc2 + 1) * 512], ot[:])


@functools.lru_cache(maxsize=1)
def _compiled():
    nc = bacc.Bacc("TRN2", target_bir_lowering=False, debug=False)
    with tile.TileContext(nc) as tc:
        emit_block(nc, tc)
    nc.compile()
    return nc


def _prepro(inputs):
    f32 = np.float32
    inp = {k: np.asarray(v, f32) for k, v in inputs.items()}
    g1, b1 = inp["ln1_g"], inp["ln1_b"]
    W = inp["attn_w"] * g1[:, None]
    bias_kqv = inp["attn_b"] + b1 @ inp["attn_w"]
    assert not np.any(bias_kqv), "nonzero attn bias not supported by this build"
    assert not np.any(inp["attn_proj_b"]) and not np.any(inp["fc_b"]) \
        and not np.any(inp["mlp_proj_b"]), "nonzero biases not supported"

    def to_f8(a):
        return np.clip(a * WS, -240.0, 240.0).astype(f8e4)

    wqk = np.ascontiguousarray(
        to_f8(W[:, :2 * C]).reshape(NCT, P, 16, P).transpose(2, 1, 0, 3))
    wv = np.ascontiguousarray(
        to_f8(W[:, 2 * C:]).reshape(NCT, P, C).transpose(1, 0, 2))
    wproj = np.ascontiguousarray(
        to_f8(inp["attn_proj_w"]).reshape(NCT, P, C).transpose(1, 0, 2))
    wfc = np.ascontiguousarray(
        (inp["fc_w"] * inp["ln2_g"][:, None]).astype(bf16)
        .reshape(NCT, P, NFT, P).transpose(2, 1, 0, 3))
    assert not np.any(inp["ln2_b"]), "nonzero ln2 bias not supported"
    wmp = np.ascontiguousarray(
        inp["mlp_proj_w"].astype(bf16).reshape(NFT, P, 2, 512).transpose(2, 1, 0, 3))
    ident = np.eye(P, dtype=bf16)
    tri01 = np.triu(np.ones((P, P), np.float32)).astype(f8e4)  # 1 where col >= row
    return inp["x"], dict(wqk=wqk, wv=wv, wproj=wproj, wfc=wfc, wmp=wmp,
                          ident=ident, tri01=tri01)


def kernel(**inputs) -> np.ndarray:
    x, weights = _prepro(inputs)
    nc = _compiled()
    in_maps = [{"x": np.ascontiguousarray(x[b]), **weights} for b in range(B)]
    res = run_bass_kernel_spmd(nc, in_maps, list(range(B)))
    return np.stack([res.results[b]["out"] for b in range(B)]).astype(np.float32)


# revision 6
# speedup vs baseline: 1.0782x; 1.0417x over previous
"""Trainium2 Bass kernel for a pre-LN transformer block (B=8,T=1024,C=1024,H=16,FF=4096).

Sharding: pure data-parallel over batch — B=8 equals the 8 NeuronCores, each core
runs the full block on one (T, C) slice; weights are replicated. No collectives.

v10 (final): token-half software pipeline on top of fp8e4-DoubleRow attention.
  - Attention is split into chunk phases: B0 processes all 16 heads on the
    i-chunk [0,512), B1 on [512,1024). During B1 the first token-half's
    attn-proj, LN2, and fc run as PE filler, overlapping the exp-paced
    attention tail. The MLP projection + second fc half follow.
  - Softmax-denominator broadcast via gpsimd partition_broadcast (no rank-1
    PE matmuls).
  - LayerNorm rstd = exp(-0.5*ln(var+eps)) on ACT: Ln and Exp share one
    activation table set with the attention exp, so the whole attention +
    LN region runs with zero ACT table switches; gelu is deferred (fc keeps
    pre-activations in SBUF) and applied in one table-set excursion pipelined
    one weight-group ahead of the MLP matmuls.
  - fp8 weights host-scaled by 32; rescales ride copy scales and
    scalar_tensor_tensor residual adds. MLP matmuls stay bf16 (fp8 there
    measured 1.7e-2 rel err, over the gate).
"""

import functools
import os

import ml_dtypes
import numpy as np

import concourse.bass as bass
import concourse.mybir as mybir
import concourse.tile as tile
from concourse import bacc
from concourse.bass_utils import run_bass_kernel_spmd

bf16 = ml_dtypes.bfloat16
f8e4 = ml_dtypes.float8_e4m3
FP32 = mybir.dt.float32
BF16 = mybir.dt.bfloat16
FP8 = mybir.dt.float8e4
AX = mybir.AxisListType
OP = mybir.AluOpType
AF = mybir.ActivationFunctionType
DR = mybir.MatmulPerfMode.DoubleRow

B, T, C, H = 8, 1024, 1024, 16
D = C // H          # 64
FF = 4 * C          # 4096
P = 128
NT = T // P         # 8 token tiles
NCT = C // P        # 8 channel tiles
NFT = FF // P       # 32 ff tiles

WS = 32.0           # fp8 weight scale (host folds WS into fp8 weights)

# Per i-chunk: (pair_vs, odd_vs) per j-tile pair; odd half of a pair must be
# zeroed on [pair_vs, odd_vs) since exp only writes [odd_vs, 512).
PAIR_VS = {0: [(0, 128), (256, 384)],
           1: [(0, 0), (0, 0), (0, 128), (256, 384)]}


def emit_block(nc, tc):
    x_d = nc.dram_tensor("x", [T, C], FP32, kind="ExternalInput").ap()
    wqk_d = nc.dram_tensor("wqk", [16, P, NCT, P], FP8, kind="ExternalInput").ap()
    wv_d = nc.dram_tensor("wv", [P, NCT, C], FP8, kind="ExternalInput").ap()
    wproj_d = nc.dram_tensor("wproj", [P, NCT, C], FP8, kind="ExternalInput").ap()
    wfc_d = nc.dram_tensor("wfc", [NFT, P, NCT, P], BF16, kind="ExternalInput").ap()
    wmp_d = nc.dram_tensor("wmp", [2, P, NFT, 512], BF16, kind="ExternalInput").ap()
    ident_d = nc.dram_tensor("ident", [P, P], BF16, kind="ExternalInput").ap()
    tri_d = nc.dram_tensor("tri01", [P, P], FP8, kind="ExternalInput").ap()
    out_d = nc.dram_tensor("out", [T, C], FP32, kind="ExternalOutput").ap()

    from contextlib import ExitStack
    with ExitStack() as top:
        cpool = top.enter_context(tc.tile_pool(name="const", bufs=1))
        ppool = top.enter_context(tc.tile_pool(name="persist", bufs=1))
        spool = top.enter_context(tc.tile_pool(name="stream", bufs=2))
        sm = top.enter_context(tc.tile_pool(name="small", bufs=4))
        aoT_pool = top.enter_context(tc.tile_pool(name="aoT", bufs=1))
        wp_pool = top.enter_context(tc.tile_pool(name="wproj", bufs=1))
        h2_pool = top.enter_context(tc.tile_pool(name="h2Tp", bufs=1))
        m0_pool = top.enter_context(tc.tile_pool(name="m0", bufs=1))
        cs = top.enter_context(ExitStack())
        ps_mm = cs.enter_context(tc.tile_pool(name="ps_mm", bufs=2, space="PSUM"))

        ident = cpool.tile([P, P], BF16, tag="ident")
        tri01 = cpool.tile([P, P], FP8, tag="tri01")
        zero1 = cpool.tile([P, 1], FP32, tag="zero1")
        eps1 = cpool.tile([P, 1], FP32, tag="eps1")

        x2_sb = ppool.tile([P, NT, C], FP32, tag="x2")
        aoT = aoT_pool.tile([P, NCT, T], FP8, tag="aoT")
        wp = wp_pool.tile([P, NCT, C], FP8, tag="wproj")
        h2T = h2_pool.tile([P, NCT, T], BF16, tag="h2T")
        m0 = m0_pool.tile([P, NFT, 512], BF16, tag="m0")  # fc half-0 pre-gelu

        def emit_ln(x_tile, h_out):
            """h_out = (x-mu)*rsqrt(var+eps). Sqrt lives in its own ACT table
            set — callers must batch LN emissions so Sqrts are consecutive in
            the ACT queue (each isolated Sqrt amid exps costs 2 table loads)."""
            width = x_tile.shape[-1]
            s = sm.tile([P, 1], FP32, tag="ln_s")
            ssq = sm.tile([P, 1], FP32, tag="ln_ssq")
            mu = sm.tile([P, 1], FP32, tag="ln_mu")
            var = sm.tile([P, 1], FP32, tag="ln_var")
            std = sm.tile([P, 1], FP32, tag="ln_std")
            rstd = sm.tile([P, 1], FP32, tag="ln_rstd")
            nc.vector.reduce_sum(s, x_tile, axis=AX.X)
            # sum-of-squares on GPSIMD (not ACT Square: the table pass assigns
            # Square a different set than Sqrt, costing 2 loads per LN; not
            # DVE so it runs concurrently with the reduce; and
            # tensor_tensor_reduce hard-faults on hw).
            # The elementwise out is scratch — write over h_out's storage.
            nc.vector.scalar_tensor_tensor(
                h_out[:, :width], x_tile, 1.0, x_tile,
                op0=OP.mult, op1=OP.mult, accum_out=ssq)
            nc.vector.tensor_scalar_mul(mu, s, 1.0 / width)
            nc.vector.tensor_scalar_mul(var, ssq, 1.0 / width)
            nc.vector.tensor_tensor(s, mu, mu, op=OP.mult)
            nc.vector.tensor_tensor(var, var, s, op=OP.subtract)
            nc.scalar.activation(std, var, AF.Sqrt, bias=1e-5)
            nc.vector.reciprocal(rstd, std)
            nc.vector.tensor_scalar(h_out, x_tile, scalar1=mu, scalar2=rstd,
                                    op0=OP.subtract, op1=OP.mult)

        with ExitStack() as attn_scope:
            ps_pv = attn_scope.enter_context(
                tc.tile_pool(name="ps_pv", bufs=2, space="PSUM"))
            ps_s = attn_scope.enter_context(
                tc.tile_pool(name="ps_s", bufs=4, space="PSUM"))
            qk_pool = attn_scope.enter_context(tc.tile_pool(name="qk", bufs=1))
            v_pool = attn_scope.enter_context(tc.tile_pool(name="v", bufs=1))
            hT_pool = attn_scope.enter_context(tc.tile_pool(name="hTp", bufs=1))
            wq_pool = attn_scope.enter_context(tc.tile_pool(name="wqkv", bufs=2))
            wv_pool = attn_scope.enter_context(tc.tile_pool(name="wvp", bufs=1))
            pt_pool = attn_scope.enter_context(tc.tile_pool(name="pt", bufs=14))
            rb_pool = attn_scope.enter_context(tc.tile_pool(name="rbp", bufs=2))

            qpT = qk_pool.tile([P, NCT, T], BF16, tag="qpT")  # q-proj^T (key role)
            kpT = qk_pool.tile([P, NCT, T], BF16, tag="kpT")  # k-proj^T (query role)
            VW = 128  # v padded to 128 cols: [0:D]=32x v, [D]=ones, rest zeros
            v_aug = v_pool.tile([P, NT, H, VW], FP8, tag="vaug")
            hT = hT_pool.tile([P, NCT, T], FP8, tag="hT")

            xts = []
            for tt in range(NT):
                xt = spool.tile([P, C], FP32, tag="xin", name=f"xin{tt}")
                nc.sync.dma_start(xt[:], x_d[tt * P:(tt + 1) * P, :])
                xts.append(xt)
            nc.sync.dma_start(ident[:], ident_d)
            nc.sync.dma_start(tri01[:], tri_d)
            nc.gpsimd.memset(zero1[:], 0.0)
            nc.gpsimd.memset(eps1[:], 1e-5)
            nc.const_aps.aps[(FP32, 0.0)] = zero1[:]
            nc.const_aps.aps[(FP32, 1e-5)] = eps1[:]
            # cols D..VW-1 are ALL ones: the PV matmul then replicates the
            # softmax denominator into psum rows 64..127, so the reciprocal
            # runs 64-lane-parallel and no partition broadcast is needed
            # (a [1,512] single-lane reciprocal measured 3.3us on DVE).
            nc.gpsimd.memset(v_aug[:, :, :, D:VW], 1.0)
            wv_sb = wv_pool.tile([P, NCT, C], FP8, tag="wv")
            nc.sync.dma_start(wv_sb[:], wv_d)
            nc.sync.dma_start(wp[:], wproj_d)

            def emit_qk_half(ft, tc2):
                wt = wq_pool.tile([P, NCT, P], FP8, tag="wqk",
                                  name=f"wqk{ft}_{tc2}")
                nc.sync.dma_start(wt[:], wqk_d[ft])
                dst, sc = (kpT, 1.0 / (WS * 8.0)) if ft < 8 else (qpT, 1.0 / WS)
                pm = ps_mm.tile([P, 512], FP32, tag="mm", name=f"qk{ft}_{tc2}")
                for c2 in range(NCT // 2):
                    nc.tensor.matmul(pm[:], wt[:, 2 * c2:2 * c2 + 2, :],
                                     hT[:, 2 * c2:2 * c2 + 2,
                                        tc2 * 512:(tc2 + 1) * 512],
                                     start=(c2 == 0), stop=(c2 == NCT // 2 - 1),
                                     perf_mode=DR)
                nc.scalar.activation(
                    dst[:, ft % 8, tc2 * 512:(tc2 + 1) * 512], pm[:],
                    AF.Copy, scale=sc)

            # ---- phase A: LN1 + transpose + v projection per token tile;
            # qk(tc2=0) halves ride along once hT tokens 0-511 exist ----
            ftlist = [v for hp in range(8) for v in (hp, 8 + hp)]
            for tt in range(NT):
                ht = spool.tile([P, C], BF16, tag="h")
                emit_ln(xts[tt][:], ht[:])
                for ct in range(NCT):
                    ptr = ps_s.tile([P, P], BF16, tag="smm")
                    nc.tensor.transpose(ptr[:], ht[:, ct * P:(ct + 1) * P], ident[:])
                    # ACT (idle here) does the psum->SBUF copies; Copy is a
                    # filler in every ACT table set so no load cost
                    nc.scalar.activation(hT[:, ct, tt * P:(tt + 1) * P],
                                         ptr[:], AF.Copy)
                for fc2 in range(2):
                    pm = ps_mm.tile([P, 512], FP32, tag="mm")
                    for c2 in range(NCT // 2):
                        nc.tensor.matmul(pm[:], hT[:, 2 * c2:2 * c2 + 2,
                                                    tt * P:(tt + 1) * P],
                                         wv_sb[:, 2 * c2:2 * c2 + 2,
                                               fc2 * 512:(fc2 + 1) * 512],
                                         start=(c2 == 0),
                                         stop=(c2 == NCT // 2 - 1),
                                         perf_mode=DR)
                    nc.scalar.activation(
                        v_aug[:, tt, fc2 * 8:(fc2 + 1) * 8, 0:D],
                        pm[:].rearrange("p (h d) -> p h d", d=D), AF.Copy)
                if tt >= 4:
                    for ft in ftlist[4 * (tt - 4):4 * (tt - 3)]:
                        emit_qk_half(ft, 0)

            # ---- attention helpers ----
            def alloc_pts(ic, h):
                out = []
                for p in range(2 * ic + 2):
                    t = pt_pool.tile([P, 2, 512], FP8, tag="pt",
                                     name=f"ptp{ic}_{h}_{p}")
                    vsp, vso = PAIR_VS[ic][p]
                    if vso > vsp:
                        nc.gpsimd.memset(t[:, 1, vsp:vso], 0.0)
                    out.append(t)
                return out

            def emit_S_one(ic, h, jt, pairs):
                po = (h % 2) * D
                cth = h // 2
                vs = max(0, jt * P - ic * 512)
                pm = ps_s.tile([P, 512], FP32, tag="smm", name=f"s{ic}_{h}_{jt}")
                nc.tensor.matmul(
                    pm[:, vs:512],
                    qpT[po:po + D, cth, jt * P:(jt + 1) * P],
                    kpT[po:po + D, cth, ic * 512 + vs:(ic + 1) * 512],
                    start=True, stop=True)
                pt = pairs[jt // 2]
                nc.scalar.activation(pt[:, jt % 2, vs:512], pm[:, vs:512], AF.Exp)
                if jt >= ic * 4:
                    dd = jt * P - ic * 512
                    # causal mask on gpsimd: pure-SBUF op, keeps it off the
                    # DVE which paces the attention phases
                    nc.gpsimd.tensor_tensor(
                        pt[:, jt % 2, dd:dd + P], pt[:, jt % 2, dd:dd + P],
                        tri01[:], op=OP.mult)

            def emit_PV(ic, h, pairs):
                pvT = ps_pv.tile([VW, 512], FP32, tag="pv", name=f"pv{ic}_{h}")
                n_p = 2 * ic + 2
                for p in range(n_p):
                    vsp = PAIR_VS[ic][p][0]
                    nc.tensor.matmul(pvT[:, vsp:512],
                                     v_aug[:, 2 * p:2 * p + 2, h, :],
                                     pairs[p][:, :, vsp:512],
                                     start=(p == 0), stop=(p == n_p - 1),
                                     perf_mode=DR)
                rd = rb_pool.tile([D, 512], BF16, tag="rd64")
                with nc.allow_low_precision(
                        reason="softmax denom recip feeds bf16 mult"):
                    nc.vector.reciprocal(rd[:], pvT[D:2 * D, :])
                return pvT, rd

            def emit_PV_norm(ic, h, pvT, rd):
                po = (h % 2) * D
                cth = h // 2
                nc.vector.tensor_tensor(
                    aoT[po:po + D, cth, ic * 512:(ic + 1) * 512],
                    pvT[0:D, :], rd[:], op=OP.mult)

            # ---- phase B0: chunk-0 attention; qk(tc2=1) halves as filler.
            # PV lags S by 2 steps and norm by 3 so exp always has slack ----
            pts_q = {}
            pv_q = {}
            B0_HEADS = [[2 * i, 2 * i + 1] for i in range(8)] + [[], []]
            for s, heads in enumerate(B0_HEADS):
                if s < 8:
                    emit_qk_half(ftlist[2 * s], 1)
                    emit_qk_half(ftlist[2 * s + 1], 1)
                prev = B0_HEADS[s - 1] if s >= 1 else []
                for h_pv in prev:
                    pv_q[h_pv] = emit_PV(0, h_pv, pts_q.pop(h_pv))
                for h in heads:
                    pts = alloc_pts(0, h)
                    for jt in range(4):
                        emit_S_one(0, h, jt, pts)
                    pts_q[h] = pts
                prev2 = B0_HEADS[s - 2] if s >= 2 else []
                for h_n in prev2:
                    emit_PV_norm(0, h_n, *pv_q.pop(h_n))

            # ---- phase B1: chunk-1 attention + token-half-0 proj/LN2/fc ----
            def emit_proj(tt):
                for cc2 in range(2):
                    pm = ps_mm.tile([P, 512], FP32, tag="mm",
                                    name=f"prj{tt}_{cc2}")
                    for c2 in range(NCT // 2):
                        nc.tensor.matmul(pm[:], aoT[:, 2 * c2:2 * c2 + 2,
                                                    tt * P:(tt + 1) * P],
                                         wp[:, 2 * c2:2 * c2 + 2,
                                            cc2 * 512:(cc2 + 1) * 512],
                                         start=(c2 == 0),
                                         stop=(c2 == NCT // 2 - 1),
                                         perf_mode=DR)
                    xr = spool.tile([P, 512], FP32, tag="xres")
                    nc.sync.dma_start(
                        xr[:], x_d[tt * P:(tt + 1) * P, cc2 * 512:(cc2 + 1) * 512])
                    nc.vector.scalar_tensor_tensor(
                        x2_sb[:, tt, cc2 * 512:(cc2 + 1) * 512], pm[:],
                        1.0 / (WS * WS), xr[:], op0=OP.mult, op1=OP.add)

            def emit_ln2(tt, trpool, trtag):
                h2 = spool.tile([P, C], BF16, tag="h")
                emit_ln(x2_sb[:, tt, :], h2[:])
                for ct in range(NCT):
                    ptr = trpool.tile([P, P], BF16, tag=trtag)
                    nc.tensor.transpose(ptr[:], h2[:, ct * P:(ct + 1) * P], ident[:])
                    nc.scalar.activation(h2T[:, ct, tt * P:(tt + 1) * P],
                                         ptr[:], AF.Copy)

            def emit_fc0(ft):
                wf = wf0_pool.tile([P, NCT, P], BF16, tag="wfc", name=f"wfc0_{ft}")
                nc.sync.dma_start(wf[:], wfc_d[ft])
                pm = ps_mm.tile([P, 512], FP32, tag="mm", name=f"fc0_{ft}")
                for ct in range(NCT):
                    nc.tensor.matmul(pm[:], wf[:, ct, :],
                                     h2T[:, ct, 0:512],
                                     start=(ct == 0), stop=(ct == NCT - 1))
                nc.scalar.activation(m0[:, ft, :], pm[:], AF.Copy)

            wf0_pool = attn_scope.enter_context(tc.tile_pool(name="wfc0", bufs=2))

            # B1 head schedule: 2 heads/step while PE is thin (attention only),
            # 1 head/step once fc chains fill the PE. proj tts 0-3 at steps
            # 0-3, LN2 burst at 4, fc half-0 chains 3/step from step 6.
            HEADS_OF = [[0, 1], [2, 3], [4, 5], [6, 7]] + \
                [[8 + i] for i in range(8)] + [[], [], []]
            fc_ft = iter(range(NFT))
            for s, heads in enumerate(HEADS_OF):
                prev = HEADS_OF[s - 1] if s >= 1 else []
                for h_pv in prev:
                    pv_q[h_pv] = emit_PV(1, h_pv, pts_q.pop(h_pv))
                fillers = []
                if 0 <= s <= 3:
                    fillers.append(('proj', s))
                if s >= 6:
                    for _ in range(3):
                        ft = next(fc_ft, None)
                        if ft is not None:
                            fillers.append(('fc', ft))
                fi = iter(fillers)

                def emit_filler(f):
                    kind, a = f
                    if kind == 'proj':
                        emit_proj(a)
                    else:
                        emit_fc0(a)

                for h in heads:
                    pts = alloc_pts(1, h)
                    for jt in range(8):
                        emit_S_one(1, h, jt, pts)
                        if jt % 3 == 2:
                            f = next(fi, None)
                            if f is not None:
                                emit_filler(f)
                    pts_q[h] = pts
                for f in fi:
                    emit_filler(f)
                if s == 4:
                    # all 4 LN2s in one burst at a step boundary: Sqrts stay
                    # consecutive in the ACT queue (2 table transitions) and
                    # don't split a head's S/exp stream
                    for tt2 in range(4):
                        emit_ln2(tt2, ps_s, "smm")
                prev2 = HEADS_OF[s - 2] if s >= 2 else []
                for h_n in prev2:
                    emit_PV_norm(1, h_n, *pv_q.pop(h_n))
            for ft in fc_ft:
                emit_fc0(ft)

        # ---- post: proj/LN2 tokens 4-7, then fc half 1 OVERLAPPED with the
        # MLP projection for token half 0 (which only needs m0); MLP half 1
        # finishes in a single ft sweep with all 8 PSUM banks ----
        if True:
            ps_fc = cs.enter_context(
                tc.tile_pool(name="ps_fc", bufs=2, space="PSUM"))
            ps_ml0 = cs.enter_context(
                tc.tile_pool(name="ps_ml0", bufs=4, space="PSUM"))
            m1_pool = cs.enter_context(tc.tile_pool(name="m1", bufs=1))
            wf_pool = cs.enter_context(tc.tile_pool(name="wfc", bufs=3))
            wm0_pool = cs.enter_context(tc.tile_pool(name="wmp0", bufs=3))
            m1 = m1_pool.tile([P, NFT, 512], BF16, tag="m1")

            # LN2 transposes for tokens 4-7 share ps_mm's bank slots
            for tt in range(4, NT):
                emit_proj(tt)
                emit_ln2(tt, ps_mm, "mm")

            # gate: 1.0, but data-dependent on the last chunk-1 norms so the
            # greedy scheduler cannot hoist the m0 gelus (and their two ACT
            # table loads) into attention-phase exp bubbles.
            gate = cpool.tile([P, 1], FP32, tag="gate")
            nc.vector.tensor_scalar(gate[:], aoT[:, NCT - 1, T - 1:T],
                                    scalar1=0.0, scalar2=1.0,
                                    op0=OP.mult, op1=OP.add)
            for fg in range(NFT // 4):
                nc.scalar.activation(m0[:, 4 * fg:4 * fg + 4, :],
                                     m0[:, 4 * fg:4 * fg + 4, :], AF.Gelu,
                                     scale=gate[:])

            def emit_fc1(ft):
                wf = wf_pool.tile([P, NCT, P], BF16, tag="wfc", name=f"wfc1_{ft}")
                nc.sync.dma_start(wf[:], wfc_d[ft])
                pm = ps_fc.tile([P, 512], FP32, tag="fcp", name=f"fc1_{ft}")
                for ct in range(NCT):
                    nc.tensor.matmul(pm[:], wf[:, ct, :],
                                     h2T[:, ct, 512:1024],
                                     start=(ct == 0), stop=(ct == NCT - 1))
                nc.scalar.activation(m1[:, ft, :], pm[:], AF.Copy)

            # fc half-1 chains interleaved with MLP-proj for token half 0.
            # mlp0 runs cc2 passes sequentially on 4 banks; 2 fc1 chains and
            # (during cc2=0) one m1 gelu group ride along per weight group.
            fc1_ft = iter(range(NFT))
            for cc2 in range(2):
                pms = [ps_ml0.tile([P, 512], FP32, tag="ml0",
                                   name=f"ml0_{cc2}_{i}") for i in range(4)]
                for fg in range(NFT // 4):
                    for _ in range(2):
                        ft1 = next(fc1_ft, None)
                        if ft1 is not None:
                            emit_fc1(ft1)
                            if ft1 % 4 == 3:
                                g = ft1 // 4
                                nc.scalar.activation(
                                    m1[:, 4 * g:4 * g + 4, :],
                                    m1[:, 4 * g:4 * g + 4, :], AF.Gelu)
                    wm = wm0_pool.tile([P, 4, 512], BF16, tag="wmp0")
                    nc.sync.dma_start(wm[:], wmp_d[cc2][:, fg * 4:(fg + 1) * 4, :])
                    for fi in range(4):
                        ft = fg * 4 + fi
                        for tt in range(4):
                            nc.tensor.matmul(pms[tt][:],
                                             m0[:, ft, tt * P:(tt + 1) * P],
                                             wm[:, fi, :],
                                             start=(ft == 0),
                                             stop=(ft == NFT - 1))
                for tt in range(4):
                    ot = spool.tile([P, 512], FP32, tag="osb")
                    nc.vector.tensor_tensor(
                        ot[:], pms[tt][:], x2_sb[:, tt, cc2 * 512:(cc2 + 1) * 512],
                        op=OP.add)
                    nc.sync.dma_start(
                        out_d[tt * P:(tt + 1) * P, cc2 * 512:(cc2 + 1) * 512],
                        ot[:])
            for ft1 in fc1_ft:
                emit_fc1(ft1)
                if ft1 % 4 == 3:
                    g = ft1 // 4
                    nc.scalar.activation(m1[:, 4 * g:4 * g + 4, :],
                                         m1[:, 4 * g:4 * g + 4, :], AF.Gelu)

            # MLP-proj token half 1 on the same 4-bank pool (a scope close
            # here costs a full-engine drain barrier mid-MLP)
            for cc2 in range(2):
                pms = [ps_ml0.tile([P, 512], FP32, tag="ml0",
                                   name=f"ml1_{cc2}_{i}") for i in range(4)]
                for fg in range(NFT // 4):
                    wm = wm0_pool.tile([P, 4, 512], BF16, tag="wmp0",
                                       name=f"wm1_{cc2}_{fg}")
                    nc.sync.dma_start(wm[:], wmp_d[cc2][:, fg * 4:(fg + 1) * 4, :])
                    for fi in range(4):
                        ft = fg * 4 + fi
                        for tt in range(4):
                            nc.tensor.matmul(pms[tt][:],
                                             m1[:, ft, tt * P:(tt + 1) * P],
                                             wm[:, fi, :],
                                             start=(ft == 0),
                                             stop=(ft == NFT - 1))
                for tt in range(4):
                    ot = spool.tile([P, 512], FP32, tag="osb")
                    nc.vector.tensor_tensor(
                        ot[:], pms[tt][:],
                        x2_sb[:, tt + 4, cc2 * 512:(cc2 + 1) * 512],
                        op=OP.add)
                    nc.sync.dma_start(
                        out_d[(tt + 4) * P:(tt + 5) * P,
                              cc2 * 512:(cc2 + 1) * 512], ot[:])


@functools.lru_cache(maxsize=1)
def _compiled():
    nc = bacc.Bacc("TRN2", target_bir_lowering=False, debug=False)
    with tile.TileContext(nc) as tc:
        emit_block(nc, tc)
    nc.compile()
    return nc


def _prepro(inputs):
    f32 = np.float32
    inp = {k: np.asarray(v, f32) for k, v in inputs.items()}
    g1, b1 = inp["ln1_g"], inp["ln1_b"]
    W = inp["attn_w"] * g1[:, None]
    bias_kqv = inp["attn_b"] + b1 @ inp["attn_w"]
    assert not np.any(bias_kqv), "nonzero attn bias not supported by this build"
    assert not np.any(inp["attn_proj_b"]) and not np.any(inp["fc_b"]) \
        and not np.any(inp["mlp_proj_b"]), "nonzero biases not supported"

    def to_f8(a):
        return np.clip(a * WS, -240.0, 240.0).astype(f8e4)

    wqk = np.ascontiguousarray(
        to_f8(W[:, :2 * C]).reshape(NCT, P, 16, P).transpose(2, 1, 0, 3))
    wv = np.ascontiguousarray(
        to_f8(W[:, 2 * C:]).reshape(NCT, P, C).transpose(1, 0, 2))
    wproj = np.ascontiguousarray(
        to_f8(inp["attn_proj_w"]).reshape(NCT, P, C).transpose(1, 0, 2))
    wfc = np.ascontiguousarray(
        (inp["fc_w"] * inp["ln2_g"][:, None]).astype(bf16)
        .reshape(NCT, P, NFT, P).transpose(2, 1, 0, 3))
    assert not np.any(inp["ln2_b"]), "nonzero ln2 bias not supported"
    wmp = np.ascontiguousarray(
        inp["mlp_proj_w"].astype(bf16).reshape(NFT, P, 2, 512).transpose(2, 1, 0, 3))
    ident = np.eye(P, dtype=bf16)
    tri01 = np.triu(np.ones((P, P), np.float32)).astype(f8e4)  # 1 where col >= row
    return inp["x"], dict(wqk=wqk, wv=wv, wproj=wproj, wfc=wfc, wmp=wmp,
                          ident=ident, tri01=tri01)


def kernel(**inputs) -> np.ndarray:
    x, weights = _prepro(inputs)
    nc = _compiled()
    in_maps = [{"x": np.ascontiguousarray(x[b]), **weights} for b in range(B)]
    res = run_bass_kernel_spmd(nc, in_maps, list(range(B)))
    return np.stack([res.results[b]["out"] for b in range(B)]).astype(np.float32)
